# revision 6
# baseline (speedup 1.0000x reference)
"""Trainium2 Bass kernel for Conf-MPU loss (nn_Conf_MPULoss).

Strategy: the loss is a streaming reduction over N rows x 5 classes down to a
handful of per-class accumulators, followed by a trivial scalar combination.

Host side:
  - rows are partitioned by label t into 5 class groups (stable), the groups
    are split evenly across 8 cores, and each per-core class segment is padded
    to a common size S = 128*R with sentinel rows. Because each device segment
    holds rows of a single known class, no t tensor and no per-class masking is
    needed on-device at all.
  - per-class counts (priors/denominators) come from a host bincount.
  - the C-length accumulators from all cores are reduced on host and combined
    into the final scalar (the "all-reduce + final combination" step).

Device side (per core, SPMD over 8 cores, identical program), per class
segment c with layout [128 partitions, R rows, 5 classes]:
    E    = exp(x)                    ScalarE (LUT)
    Z    = sum_classes(E)            VectorE tensor_reduce (single-input op:
                                     uses DVE's dedicated SBUF port only)
    lnZ  = ln(Z)                     ScalarE (same LUT set as exp: the
                                     activation-table map is restricted so
                                     exp/ln resolve to natural_log_exp_and_
                                     others -> one ACT_TABLE_LOAD total)
    c < 4:
      dt  = lnZ - xc  (= -log p_c)   DVE STT, fused accum -> Sa_c
      d4  = lnZ - x4  (= -log p_neg) DVE STT, fused accum -> Sb_c
      den = sum(sign(ln2 - dt))      ScalarE Sign with fused accum
                                     (den_c = (acc + S)/2 on host)
      edt = exp(dt)   (= 1/p_c)      ScalarE
      u   = edt * d4                 GpSimd TT (keeps DVE free; GpSimd only
                                     supports plain tensor_tensor)
      num accum: sum((dt < ln2) * u) DVE STT, fused accum -> num_c
    c == 4:
      d4   = lnZ - x4                DVE STT, fused accum (unused col)
      xm   = max_classes(x)          VectorE tensor_reduce
      dmin = lnZ - xm (= -log p_max) GpSimd TT
      li accum: sum((dmin >= ln2) * d4)  DVE STT, fused accum -> li

Key identities used:
  risk1 - risk3 needs only Sa_c - Sb_c = sum_{t=c}(x4 - xc): the lnZ terms
  cancel, so the sentinel-pad contribution (exactly 20.0 per pad row) can be
  subtracted exactly on the host.
  p_c > 0.5  <=>  dt < ln2;  all p <= 0.5  <=>  lnZ - max(x) >= ln2.
  exp without max-subtraction is fp32-safe because inputs are O(1) logits.
"""

import numpy as np

import concourse.bacc as bacc
import concourse.mybir as mybir
import concourse.tile as tile
from concourse import bass_utils

F32 = mybir.dt.float32
Alu = mybir.AluOpType
Act = mybir.ActivationFunctionType

LN2 = 0.6931471805599453
P = 128
NCLS = 5
N_CORES = 8
NCOLS = 4 * NCLS  # per segment c: [Sb, Sa|li, den, num]

# pad sentinel row: classes 0..3 = -10, class 4 = +10.
# For a class-c (c<4) segment pad row: dt ~ 20 (no num/den hit, Sa-Sb hit of
# exactly 20.0 each, corrected on host); for the class-4 segment dmin ~ 0 so
# the li mask is 0.
PAD_ROW = np.array([-10.0, -10.0, -10.0, -10.0, 10.0], dtype=np.float32)

_PROGRAM_CACHE: dict[int, tuple] = {}


def _restrict_act_tables(arch: str):
    """Confine Exp/Ln to the natural_log_exp_and_others set so the act-table
    pass emits a single ACT_TABLE_LOAD instead of thrashing between the
    exp_and_others and natural_log sets (~1.3us per load)."""
    from concourse import hw_specs

    tables = hw_specs.get_activation_tables(arch)
    if "natural_log_exp_and_others" not in tables:
        return
    for name, funcs in tables.items():
        if name != "natural_log_exp_and_others":
            funcs.discard(Act.Exp)
            funcs.discard(Act.Ln)


def _build_program(R: int):
    """Build + compile the per-core Bass program for segment length S=128*R."""
    nc = bacc.Bacc("TRN2", debug=False, num_devices=N_CORES)
    _restrict_act_tables(nc.m.arch)
    # register a const AP for the ln2 activation bias (only 0.0/1.0 exist)
    _ct = nc.alloc_sbuf_tensor(f"const-float32-{LN2}", [128, 1], F32)
    nc.gpsimd.memset(_ct.ap(), LN2)
    nc.const_aps.aps[(F32, LN2)] = _ct.ap()
    nc.all_engine_barrier()
    x_d = nc.dram_tensor("x", [NCLS, P, R * 5], F32, kind="ExternalInput").ap()
    st_d = nc.dram_tensor("stats", [P, NCOLS], F32, kind="ExternalOutput").ap()

    with tile.TileContext(nc) as tc:
        with (
            tc.tile_pool(name="io", bufs=2) as iop,
            tc.tile_pool(name="wk", bufs=2) as wp,
            tc.tile_pool(name="st", bufs=1) as sp,
        ):
            stats = sp.tile([P, NCOLS], F32)
            nc.vector.memset(stats, 0.0)
            for c in range(NCLS):
                X = iop.tile([P, R * 5], F32, tag="x")
                nc.sync.dma_start(out=X, in_=x_d[c])
                Xv = X.rearrange("p (r c) -> p r c", c=5)
                E = wp.tile([P, R * 5], F32, tag="e")
                nc.scalar.activation(E, X, Act.Exp)
                Z = wp.tile([P, R], F32, tag="z")
                nc.vector.tensor_reduce(
                    Z,
                    E.rearrange("p (r c) -> p r c", c=5),
                    axis=mybir.AxisListType.X,
                    op=Alu.add,
                )
                LnZ = wp.tile([P, R], F32, tag="lnz")
                nc.scalar.activation(LnZ, Z, Act.Ln)
                D4 = wp.tile([P, R], F32, tag="d4")
                nc.vector.scalar_tensor_tensor(
                    out=D4,
                    in0=Xv[:, :, 4],
                    scalar=-1.0,
                    in1=LnZ,
                    op0=Alu.mult,
                    op1=Alu.add,
                    accum_out=stats[:, 4 * c + 0 : 4 * c + 1],
                )
                if c < 4:
                    DT = wp.tile([P, R], F32, tag="dt")
                    nc.vector.scalar_tensor_tensor(
                        out=DT,
                        in0=Xv[:, :, c],
                        scalar=-1.0,
                        in1=LnZ,
                        op0=Alu.mult,
                        op1=Alu.add,
                        accum_out=stats[:, 4 * c + 1 : 4 * c + 2],
                    )
                    # den_c: sum(sign(ln2 - dt)) on ScalarE; host: (acc+S)/2
                    SJ = wp.tile([P, R], F32, tag="sj")
                    nc.scalar.activation(
                        SJ,
                        DT,
                        Act.Sign,
                        scale=-1.0,
                        bias=LN2,
                        accum_out=stats[:, 4 * c + 2 : 4 * c + 3],
                    )
                    EDT = wp.tile([P, R], F32, tag="edt")
                    nc.scalar.activation(EDT, DT, Act.Exp)
                    U = wp.tile([P, R], F32, tag="u")
                    nc.gpsimd.tensor_tensor(out=U, in0=EDT, in1=D4, op=Alu.mult)
                    G = wp.tile([P, R], F32, tag="g")
                    nc.vector.scalar_tensor_tensor(
                        out=G,
                        in0=DT,
                        scalar=LN2,
                        in1=U,
                        op0=Alu.is_lt,
                        op1=Alu.mult,
                        accum_out=stats[:, 4 * c + 3 : 4 * c + 4],
                    )
                else:
                    XM = wp.tile([P, R], F32, tag="xm")
                    nc.vector.tensor_reduce(
                        XM, Xv, axis=mybir.AxisListType.X, op=Alu.max
                    )
                    DMIN = wp.tile([P, R], F32, tag="dmin")
                    nc.gpsimd.tensor_tensor(
                        out=DMIN, in0=LnZ, in1=XM, op=Alu.subtract
                    )
                    G = wp.tile([P, R], F32, tag="g")
                    nc.vector.scalar_tensor_tensor(
                        out=G,
                        in0=DMIN,
                        scalar=LN2,
                        in1=D4,
                        op0=Alu.is_ge,
                        op1=Alu.mult,
                        accum_out=stats[:, 4 * c + 1 : 4 * c + 2],
                    )
            nc.sync.dma_start(out=st_d, in_=stats)
    nc.compile()
    return nc


def _get_program(R: int):
    if R not in _PROGRAM_CACHE:
        _PROGRAM_CACHE[R] = _build_program(R)
    return _PROGRAM_CACHE[R]


def _prepare_inputs(x: np.ndarray, t: np.ndarray):
    """Sort rows by class, shard across cores, pad segments. Returns
    (in_maps, counts, n_pad_per_class_total, R)."""
    N = x.shape[0]
    t64 = t.astype(np.int64, copy=False)
    counts = np.bincount(t64, minlength=NCLS).astype(np.int64)

    # per-core per-class row counts (even split of each class across cores)
    n_ck = np.zeros((NCLS, N_CORES), dtype=np.int64)
    for c in range(NCLS):
        q, r = divmod(int(counts[c]), N_CORES)
        n_ck[c] = q
        n_ck[c, :r] += 1

    R = int(max(8, -(-int(n_ck.max()) // P)))
    R = (R + 1) // 2 * 2  # keep it even
    S = P * R

    order = np.argsort(t64, kind="stable")
    xs = np.ascontiguousarray(x[order], dtype=np.float32)
    starts = np.concatenate([[0], np.cumsum(counts)])

    xcores = np.empty((N_CORES, NCLS, S, 5), dtype=np.float32)
    xcores[:] = PAD_ROW
    for c in range(NCLS):
        off = int(starts[c])
        for k in range(N_CORES):
            n = int(n_ck[c, k])
            if n:
                xcores[k, c, :n] = xs[off : off + n]
                off += n

    in_maps = [{"x": xcores[k].reshape(NCLS, P, R * 5)} for k in range(N_CORES)]
    n_pad = N_CORES * S - counts  # per class, summed over cores
    return in_maps, counts, n_pad, R


def _combine(stats_list, counts, n_pad, N, R):
    """Host all-reduce of the C-length accumulators + final scalar combination."""
    st = np.zeros(NCOLS, dtype=np.float64)
    for s in stats_list:
        st += s.astype(np.float64).sum(axis=0)

    S_total = N_CORES * P * R  # rows per class segment, summed over cores
    counts = counts.astype(np.float64)
    r13 = 0.0  # risk1 - risk3
    r2 = 0.0
    for c in range(4):
        sb = st[4 * c + 0]
        sa = st[4 * c + 1]
        den = (st[4 * c + 2] + S_total) / 2.0  # sign-sum -> count
        num = st[4 * c + 3]
        sd = (sa - sb) - 20.0 * float(n_pad[c])  # sum_{t=c}(x4 - xc), pads removed
        prior = counts[c] / N
        r13 += prior * sd / max(1.0, counts[c])
        r2 += prior * num / max(den, 1.0)
    li = st[4 * 4 + 1]
    r4 = li / max(1.0, counts[4])

    pos = 4.0 * (r13 + r2)
    if pos < 0.0:
        pos = 0.0
    return np.float32(pos + r4)


def run_device(in_maps, R, trace=False, **kw):
    nc = _get_program(R)
    res = bass_utils.run_bass_kernel_spmd(
        nc, in_maps, core_ids=list(range(N_CORES)), trace=trace, **kw
    )
    return res


def kernel(x: np.ndarray, t: np.ndarray) -> np.ndarray:
    x = np.asarray(x, dtype=np.float32)
    t = np.asarray(t)
    N = x.shape[0]
    in_maps, counts, n_pad, R = _prepare_inputs(x, t)
    res = run_device(in_maps, R)
    stats_list = [res.results[k]["stats"] for k in range(N_CORES)]
    return _combine(stats_list, counts, n_pad, N, R)


# revision 9
# speedup vs baseline: 1.2169x; 1.2169x over previous
"""Trainium2 Bass kernel for Conf-MPU loss (nn_Conf_MPULoss).

Strategy: the loss is a streaming reduction over N rows x 5 classes down to a
handful of per-class accumulators, followed by a trivial scalar combination.

Host side:
  - rows are partitioned by label t into 5 class groups (stable), the groups
    are split evenly across 8 cores, and each per-core class segment is padded
    to a common size S = 128*R with sentinel rows. Because each device segment
    holds rows of a single known class, no t tensor and no per-class masking is
    needed on-device at all.
  - per-class counts (priors/denominators) come from a host bincount.
  - the C-length accumulators from all cores are reduced on host and combined
    into the final scalar (the "all-reduce + final combination" step).

Device side (per core, SPMD over 8 cores, identical program), per class
segment c with layout [128 partitions, R rows, 5 classes]:
    E    = exp(x)                    ScalarE (LUT)
    Z    = sum_classes(E)            VectorE tensor_reduce (single-input op:
                                     uses DVE's dedicated SBUF port only)
    lnZ  = ln(Z)                     ScalarE (same LUT set as exp: the
                                     activation-table map is restricted so
                                     exp/ln resolve to natural_log_exp_and_
                                     others -> one ACT_TABLE_LOAD total)
    c < 4:
      dt  = lnZ - xc  (= -log p_c)   DVE STT, fused accum -> Sa_c
      d4  = lnZ - x4  (= -log p_neg) DVE STT, fused accum -> Sb_c
      den = sum(sign(ln2 - dt))      ScalarE Sign with fused accum
                                     (den_c = (acc + S)/2 on host)
      edt = exp(dt)   (= 1/p_c)      ScalarE
      u   = edt * d4                 GpSimd TT (keeps DVE free; GpSimd only
                                     supports plain tensor_tensor)
      num accum: sum((dt < ln2) * u) DVE STT, fused accum -> num_c
    c == 4:
      d4   = lnZ - x4                DVE STT, fused accum (unused col)
      xm   = max_classes(x)          VectorE tensor_reduce
      dmin = lnZ - xm (= -log p_max) GpSimd TT
      li accum: sum((dmin >= ln2) * d4)  DVE STT, fused accum -> li

Key identities used:
  risk1 - risk3 needs only Sa_c - Sb_c = sum_{t=c}(x4 - xc): the lnZ terms
  cancel, so the sentinel-pad contribution (exactly 20.0 per pad row) can be
  subtracted exactly on the host.
  p_c > 0.5  <=>  dt < ln2;  all p <= 0.5  <=>  lnZ - max(x) >= ln2.
  exp without max-subtraction is fp32-safe because inputs are O(1) logits.
"""

import numpy as np

import concourse.bacc as bacc
import concourse.mybir as mybir
import concourse.tile as tile
from concourse import bass_utils

F32 = mybir.dt.float32
Alu = mybir.AluOpType
Act = mybir.ActivationFunctionType

LN2 = 0.6931471805599453
P = 128
NCLS = 5
N_CORES = 8
NCOLS = 4 * NCLS  # per segment c: [Sb, Sa|li, den, num]

# pad sentinel row: classes 0..3 = -10, class 4 = +10.
# For a class-c (c<4) segment pad row: dt ~ 20 (no num/den hit, Sa-Sb hit of
# exactly 20.0 each, corrected on host); for the class-4 segment dmin ~ 0 so
# the li mask is 0.
PAD_ROW = np.array([-10.0, -10.0, -10.0, -10.0, 10.0], dtype=np.float32)

_PROGRAM_CACHE: dict[int, tuple] = {}


def _restrict_act_tables(arch: str):
    """Confine Exp/Ln to the natural_log_exp_and_others set so the act-table
    pass emits a single ACT_TABLE_LOAD instead of thrashing between the
    exp_and_others and natural_log sets (~1.3us per load)."""
    from concourse import hw_specs

    tables = hw_specs.get_activation_tables(arch)
    if "natural_log_exp_and_others" not in tables:
        return
    for name, funcs in tables.items():
        if name != "natural_log_exp_and_others":
            funcs.discard(Act.Exp)
            funcs.discard(Act.Ln)


def _build_program(R: int):
    """Build + compile the per-core Bass program for segment length S=128*R."""
    nc = bacc.Bacc("TRN2", debug=False, num_devices=N_CORES)
    _restrict_act_tables(nc.m.arch)
    # register a const AP for the ln2 activation bias (only 0.0/1.0 exist)
    _ct = nc.alloc_sbuf_tensor(f"const-float32-{LN2}", [128, 1], F32)
    nc.gpsimd.memset(_ct.ap(), LN2)
    nc.const_aps.aps[(F32, LN2)] = _ct.ap()
    nc.all_engine_barrier()
    x_d = nc.dram_tensor("x", [NCLS, P, R * 5], F32, kind="ExternalInput").ap()
    st_d = nc.dram_tensor("stats", [P, NCOLS], F32, kind="ExternalOutput").ap()

    with tile.TileContext(nc) as tc:
        with (
            tc.tile_pool(name="io", bufs=NCLS) as iop,
            tc.tile_pool(name="wk", bufs=2) as wp,
            tc.tile_pool(name="st", bufs=1) as sp,
        ):
            stats = sp.tile([P, NCOLS], F32)
            nc.vector.memset(stats, 0.0)
            # segment 4 first: its chain (sum+max reduces) is the longest
            for c in (4, 0, 1, 2, 3):
                X = iop.tile([P, R * 5], F32, tag="x")
                nc.sync.dma_start(out=X, in_=x_d[c])
                Xv = X.rearrange("p (r c) -> p r c", c=5)
                E = wp.tile([P, R * 5], F32, tag="e")
                nc.scalar.activation(E, X, Act.Exp)
                Z = wp.tile([P, R], F32, tag="z")
                nc.vector.tensor_reduce(
                    Z,
                    E.rearrange("p (r c) -> p r c", c=5),
                    axis=mybir.AxisListType.X,
                    op=Alu.add,
                )
                LnZ = wp.tile([P, R], F32, tag="lnz")
                nc.scalar.activation(LnZ, Z, Act.Ln)
                D4 = wp.tile([P, R], F32, tag="d4")
                nc.vector.scalar_tensor_tensor(
                    out=D4,
                    in0=Xv[:, :, 4],
                    scalar=-1.0,
                    in1=LnZ,
                    op0=Alu.mult,
                    op1=Alu.add,
                    accum_out=stats[:, 4 * c + 0 : 4 * c + 1],
                )
                if c < 4:
                    DT = wp.tile([P, R], F32, tag="dt")
                    nc.vector.scalar_tensor_tensor(
                        out=DT,
                        in0=Xv[:, :, c],
                        scalar=-1.0,
                        in1=LnZ,
                        op0=Alu.mult,
                        op1=Alu.add,
                        accum_out=stats[:, 4 * c + 1 : 4 * c + 2],
                    )
                    # den_c: sum(sign(ln2 - dt)) on ScalarE; host: (acc+S)/2
                    SJ = wp.tile([P, R], F32, tag="sj")
                    nc.scalar.activation(
                        SJ,
                        DT,
                        Act.Sign,
                        scale=-1.0,
                        bias=LN2,
                        accum_out=stats[:, 4 * c + 2 : 4 * c + 3],
                    )
                    EDT = wp.tile([P, R], F32, tag="edt")
                    nc.scalar.activation(EDT, DT, Act.Exp)
                    U = wp.tile([P, R], F32, tag="u")
                    nc.gpsimd.tensor_tensor(out=U, in0=EDT, in1=D4, op=Alu.mult)
                    G = wp.tile([P, R], F32, tag="g")
                    nc.vector.scalar_tensor_tensor(
                        out=G,
                        in0=DT,
                        scalar=LN2,
                        in1=U,
                        op0=Alu.is_lt,
                        op1=Alu.mult,
                        accum_out=stats[:, 4 * c + 3 : 4 * c + 4],
                    )
                else:
                    XM = wp.tile([P, R], F32, tag="xm")
                    nc.vector.tensor_reduce(
                        XM, Xv, axis=mybir.AxisListType.X, op=Alu.max
                    )
                    DMIN = wp.tile([P, R], F32, tag="dmin")
                    nc.gpsimd.tensor_tensor(
                        out=DMIN, in0=LnZ, in1=XM, op=Alu.subtract
                    )
                    G = wp.tile([P, R], F32, tag="g")
                    nc.vector.scalar_tensor_tensor(
                        out=G,
                        in0=DMIN,
                        scalar=LN2,
                        in1=D4,
                        op0=Alu.is_ge,
                        op1=Alu.mult,
                        accum_out=stats[:, 4 * c + 1 : 4 * c + 2],
                    )
            nc.sync.dma_start(out=st_d, in_=stats)
    nc.compile()
    return nc


def _get_program(R: int):
    if R not in _PROGRAM_CACHE:
        _PROGRAM_CACHE[R] = _build_program(R)
    return _PROGRAM_CACHE[R]


def _prepare_inputs(x: np.ndarray, t: np.ndarray):
    """Sort rows by class, shard across cores, pad segments. Returns
    (in_maps, counts, n_pad_per_class_total, R)."""
    N = x.shape[0]
    t64 = t.astype(np.int64, copy=False)
    counts = np.bincount(t64, minlength=NCLS).astype(np.int64)

    # per-core per-class row counts (even split of each class across cores)
    n_ck = np.zeros((NCLS, N_CORES), dtype=np.int64)
    for c in range(NCLS):
        q, r = divmod(int(counts[c]), N_CORES)
        n_ck[c] = q
        n_ck[c, :r] += 1

    R = int(max(8, -(-int(n_ck.max()) // P)))
    R = (R + 1) // 2 * 2  # keep it even
    S = P * R

    order = np.argsort(t64, kind="stable")
    xs = np.ascontiguousarray(x[order], dtype=np.float32)
    starts = np.concatenate([[0], np.cumsum(counts)])

    xcores = np.empty((N_CORES, NCLS, S, 5), dtype=np.float32)
    xcores[:] = PAD_ROW
    for c in range(NCLS):
        off = int(starts[c])
        for k in range(N_CORES):
            n = int(n_ck[c, k])
            if n:
                xcores[k, c, :n] = xs[off : off + n]
                off += n

    in_maps = [{"x": xcores[k].reshape(NCLS, P, R * 5)} for k in range(N_CORES)]
    n_pad = N_CORES * S - counts  # per class, summed over cores
    return in_maps, counts, n_pad, R


def _combine(stats_list, counts, n_pad, N, R):
    """Host all-reduce of the C-length accumulators + final scalar combination."""
    st = np.zeros(NCOLS, dtype=np.float64)
    for s in stats_list:
        st += s.astype(np.float64).sum(axis=0)

    S_total = N_CORES * P * R  # rows per class segment, summed over cores
    counts = counts.astype(np.float64)
    r13 = 0.0  # risk1 - risk3
    r2 = 0.0
    for c in range(4):
        sb = st[4 * c + 0]
        sa = st[4 * c + 1]
        den = (st[4 * c + 2] + S_total) / 2.0  # sign-sum -> count
        num = st[4 * c + 3]
        sd = (sa - sb) - 20.0 * float(n_pad[c])  # sum_{t=c}(x4 - xc), pads removed
        prior = counts[c] / N
        r13 += prior * sd / max(1.0, counts[c])
        r2 += prior * num / max(den, 1.0)
    li = st[4 * 4 + 1]
    r4 = li / max(1.0, counts[4])

    pos = 4.0 * (r13 + r2)
    if pos < 0.0:
        pos = 0.0
    return np.float32(pos + r4)


def run_device(in_maps, R, trace=False, **kw):
    nc = _get_program(R)
    res = bass_utils.run_bass_kernel_spmd(
        nc, in_maps, core_ids=list(range(N_CORES)), trace=trace, **kw
    )
    return res


def kernel(x: np.ndarray, t: np.ndarray) -> np.ndarray:
    x = np.asarray(x, dtype=np.float32)
    t = np.asarray(t)
    N = x.shape[0]
    in_maps, counts, n_pad, R = _prepare_inputs(x, t)
    res = run_device(in_maps, R)
    stats_list = [res.results[k]["stats"] for k in range(N_CORES)]
    return _combine(stats_list, counts, n_pad, N, R)
